# revision 37
# baseline (speedup 1.0000x reference)
"""Multi-head attention (B=2, S=2048, D=1024, H=16, causal) on 8 TRN2 NeuronCores.

Sharding: 8 cores = 2 batches x 4 head-groups (4 heads each).  Each core
computes the QKV projections for its head slice, causal attention for its 4
heads, and the partial output projection (input-dim slice of Wo).  The
all-reduce over head-groups happens at gather time on the host (sum of 4
partials per batch).

Everything on device works in token-transposed layout ([feature, token]):
  scores^T[kv, q] = K_projT_tile^T @ Q_projT   (K = dh = 64)
  P = exp(scores^T)  (no max subtraction: scores ~ N(0,1), |s| < ~7)
  out^T = [ones|V|ones] slices^T @ P           (ones half -> softmax denom,
                                                replicated across 64 rows)
  partial^T[dmodel, tok] = WoT_slice^T @ attn_out^T

v3 restructure (from trace analysis of the 184.7us v2):
  - softmax denominator replicated across 64 PSUM partitions by the PE
    itself: AV stationary is a 192-wide [ones|V|ones] band, sliced
    [64:192] for even heads (av rows 0..63, denom rows 64..127) and
    [0:128] for odd heads (denom rows 0..63, av rows 64..127).  M=128 vs
    65 costs zero extra PE cycles.  normalize is then: one PSUM->SBUF
    evacuation copy, two reciprocal_approx_fast, two 64-partition shift
    DMAs on the idle sync ring, two aligned DVE multiplies straight into
    attn2.  No gpsimd partition_broadcast / DMA round trips (v2's 10.6us
    tail stall).
  - q/k/v projection PSUM->SBUF copies moved from DVE to the scalar
    engine (Copy lives in every ACT table set; ACT is idle during the
    projection phase).  DVE keeps mask muls, oproj copies, normalize.
  - startup: tiny warm memset is the FIRST gpsimd op (warmup matmuls
    start ~1.5us, was 12.5us); wk/k0 DMAs move to the scalar ring;
    vproj ones-memsets run on DVE.
"""

import math
import os

import numpy as np
import ml_dtypes

_BF16 = ml_dtypes.bfloat16

B, S, D = 2, 2048, 1024
H, DH = 16, 64
NCORES = 8
GRP = 4  # heads per core
KT = D // 128  # 8 k-tiles over d_model
NQ = 512  # q tile width
QTILES = S // NQ  # 4
KVTILES = S // 128  # 16

last_results = None

_programs = {}


def _build_program(causal: bool):
    import concourse.bass as bass
    import concourse.mybir as mybir
    import concourse.tile as tile
    from concourse import bacc

    f32 = mybir.dt.float32
    bf16 = mybir.dt.bfloat16
    Exp = mybir.ActivationFunctionType.Exp

    nc = bacc.Bacc(
        "TRN2",
        target_bir_lowering=False,
        debug=False,
        enable_asserts=False,
        num_devices=NCORES,
    )

    # all inputs host-pre-tiled so every DMA is one instruction with >=8KB
    # contiguous per-partition lines (1KB lines are packet-bound at ~97GB/s)
    qT = nc.dram_tensor("qT", [QTILES, 128, KT * NQ], bf16, kind="ExternalInput").ap()
    kT = nc.dram_tensor("kT", [QTILES, 128, KT * NQ], bf16, kind="ExternalInput").ap()
    vT = nc.dram_tensor("vT", [QTILES, 128, KT * NQ], bf16, kind="ExternalInput").ap()
    wqT = nc.dram_tensor("wqT", [128, KT * 256], bf16, kind="ExternalInput").ap()
    wkT = nc.dram_tensor("wkT", [128, KT * 256], bf16, kind="ExternalInput").ap()
    wvT = nc.dram_tensor("wvT", [128, KT * 256], bf16, kind="ExternalInput").ap()
    woT = nc.dram_tensor("woT", [128, 2 * D], bf16, kind="ExternalInput").ap()
    if not causal:
        maskT = nc.dram_tensor("maskT", [S, S], bf16, kind="ExternalInput").ap()
    out = nc.dram_tensor("out", [D, S], bf16, kind="ExternalOutput").ap()

    with tile.TileContext(nc) as tc:
        with (
            tc.tile_pool(name="persist", bufs=1) as sb,
            tc.tile_pool(name="stream", bufs=3) as stream,
            tc.tile_pool(name="psum", bufs=1, space="PSUM") as psum,
            tc.tile_pool(name="p_sb", bufs=6) as pbuf,
            tc.tile_pool(name="r_sb", bufs=2) as rpool,
            tc.tile_pool(name="m_sb", bufs=4) as mpool,
            tc.tile_pool(name="o_sb", bufs=4) as opool,
        ):
            # ---- persistent SBUF tensors ----
            wq_sb = sb.tile([128, KT, 256], bf16)
            wk_sb = sb.tile([128, KT, 256], bf16)
            wv_sb = sb.tile([128, KT, 256], bf16)
            wo2 = sb.tile([128, 2, D], bf16)  # head h at rows 64*(h%2), chunk h//2
            qproj = sb.tile([128, 2, S], bf16)
            kproj = sb.tile([128, 2, S], bf16)
            attn2 = sb.tile([128, 2, S], bf16)  # head h at rows 64*(h%2), chunk h//2

            # V with 64-wide ones flanks: per (kv-tile, head g): cols 0..63 =
            # ones, 64..127 = V, 128..191 = ones.  Stationary [64:192] ->
            # av rows 0..63 + denom replicated on rows 64..127 (even heads);
            # [0:128] -> denom replicated rows 0..63 + av rows 64..127 (odd
            # heads).  Replication is free on the PE (M=128 vs 65 costs no
            # cycles) and lets the tail chain run a direct lane-parallel
            # reciprocal with no DMA round trip.
            vproj = sb.tile([128, KVTILES, GRP, 192], bf16)

            # ---- gpsimd queue: tiny warm memset FIRST so warmup matmuls
            # are unblocked right after the engine preamble (~5us).
            warm = sb.tile([128, 256], bf16)
            nc.gpsimd.memset(warm[:], 0.0)

            # PE warmup: dummy matmuls fill the DMA-paced startup window and
            # keep the clock ramp going until real matmuls start.
            _warm_ctr = [0]

            def warmup(k):
                for _ in range(k):
                    w = _warm_ctr[0]
                    _warm_ctr[0] += 1
                    wp = psum.tile(
                        [128, 256], f32, tag="mm", bufs=2, name=f"warm{w}"
                    )
                    nc.tensor.matmul(
                        wp[:], warm[:, 0:128], warm[:], start=True, stop=True
                    )

            # ---- input DMA prologue.  Aggregate DMA bandwidth is ~280GB/s
            # shared across the three rings (sync HWDGE, scalar HWDGE,
            # gpsimd SWDGE), so what matters is global need-order: wq+q0
            # first (first projection at ~12us), then k0+wk, wv+v0, then the
            # later groups.  Each 1MB token group is split into two 0.5MB
            # halves on different rings to keep arrival granularity fine.
            qg, kg, vg = [], [], []

            def _mktile(dst_list, tag):
                n = len(dst_list)
                t = stream.tile(
                    [128, KT, NQ], bf16, tag=tag, bufs=3, name=f"{tag}{n}"
                )
                dst_list.append(t)
                return t

            def _half(t, src, n, half, eng):
                h = KT // 2
                sl = slice(half * h, (half + 1) * h)
                eng.dma_start(t[:, sl, :], src[n][:, half * h * NQ : (half + 1) * h * NQ])

            # global need order: wq, q0 | wk, k0 | v0, q1 | k1, v1 | q2, k2 |
            # q3, k3 | v2, v3 (wo anywhere past ~30us).  q0/k0 are halved
            # across sync+gpsimd so the first projections start ~12us; the
            # rest are full 1MB transfers (fewer completion bubbles on the 8
            # DMA semaphore lanes).  Ring FIFO order matches need order.
            nc.scalar.dma_start(wq_sb[:], wqT[:])
            t = _mktile(qg, "qg")
            _half(t, qT, 0, 0, nc.sync)
            _half(t, qT, 0, 1, nc.gpsimd)
            t = _mktile(kg, "kg")
            _half(t, kT, 0, 0, nc.sync)
            _half(t, kT, 0, 1, nc.gpsimd)
            nc.scalar.dma_start(wk_sb[:], wkT[:])
            nc.scalar.dma_start(wv_sb[:], wvT[:])
            t = _mktile(vg, "vg")
            nc.sync.dma_start(t[:], vT[0])
            t = _mktile(qg, "qg")
            nc.gpsimd.dma_start(t[:], qT[1])
            t = _mktile(kg, "kg")
            nc.sync.dma_start(t[:], kT[1])
            t = _mktile(vg, "vg")
            nc.sync.dma_start(t[:], vT[1])
            t = _mktile(qg, "qg")
            nc.gpsimd.dma_start(t[:], qT[2])
            t = _mktile(kg, "kg")
            nc.gpsimd.dma_start(t[:], kT[2])
            nc.scalar.dma_start(wo2[:], woT[:])
            t = _mktile(qg, "qg")
            nc.sync.dma_start(t[:], qT[3])
            t = _mktile(kg, "kg")
            nc.gpsimd.dma_start(t[:], kT[3])
            t = _mktile(vg, "vg")
            nc.gpsimd.dma_start(t[:], vT[2])
            t = _mktile(vg, "vg")
            nc.sync.dma_start(t[:], vT[3])

            # gpsimd setup after its DMA issues: causal mask (needed ~28us)
            if causal:
                # single 128x128 causal block: keep where q_local >= kv_local
                mask128 = sb.tile([128, 128], bf16)
                nc.gpsimd.memset(mask128[:], 1.0)
                nc.gpsimd.affine_select(
                    out=mask128[:],
                    in_=mask128[:],
                    compare_op=mybir.AluOpType.is_ge,
                    fill=0.0,
                    base=0,
                    pattern=[[1, 128]],
                    channel_multiplier=-1,
                )
            # ones flanks on gpsimd (its queue is idle once the DMA
            # issues are out; done by ~20us, first AV needs them ~28us)
            nc.gpsimd.memset(vproj[:, :, :, 0:64], 1.0)
            nc.gpsimd.memset(vproj[:, :, :, 128:192], 1.0)

            warmup(24)

            def qkproj(which, m2, n):
                w_sb = wq_sb if which == "q" else wk_sb
                xt = (qg if which == "q" else kg)[n]
                proj = qproj if which == "q" else kproj
                ps = psum.tile([128, NQ], f32, tag="mm", bufs=2)
                for kt in range(KT):
                    nc.tensor.matmul(
                        ps[:],
                        w_sb[:, kt, 128 * m2 : 128 * m2 + 128],
                        xt[:, kt, :],
                        start=(kt == 0),
                        stop=(kt == KT - 1),
                    )
                nc.scalar.copy(proj[:, m2, NQ * n : NQ * n + NQ], ps[:])

            def vproj_tile(mt):
                vt = vg[mt // 4]
                col = 128 * (mt % 4)
                ps = psum.tile([128, 256], f32, tag="mm", bufs=2)
                for kt in range(KT):
                    nc.tensor.matmul(
                        ps[:],
                        vt[:, kt, col : col + 128],
                        wv_sb[:, kt, :],
                        start=(kt == 0),
                        stop=(kt == KT - 1),
                    )
                nc.scalar.copy(
                    vproj[:, mt, :, 64:128],
                    ps[:].rearrange("p (h d) -> p h d", h=GRP),
                )

            # attention state per (c2): av psum tile + per-tile units
            def attn_units(c2, j, avt):
                """Yield per-kv-tile unit emitters for head pair c2, q-tile j."""
                ktiles = 4 * j + 4 if causal else KVTILES
                p2s = {}

                def sc_exp(t):
                    d = t - 4 * j
                    off = 128 * d if (causal and d >= 0) else 0
                    sc = psum.tile([128, 2, NQ], f32, tag="sc", bufs=2)
                    for i in range(2):
                        base = 64 * i
                        nc.tensor.matmul(
                            sc[:, i, off:NQ],
                            kproj[base : base + 64, c2, 128 * t : 128 * t + 128],
                            qproj[base : base + 64, c2, NQ * j + off : NQ * j + NQ],
                            start=True,
                            stop=True,
                        )
                    p2 = pbuf.tile([128, 2, NQ], bf16, tag="p")
                    p2s[t] = p2
                    nc.scalar.activation(p2[:, :, off:NQ], sc[:, :, off:NQ], Exp)

                def mask_av(t):
                    d = t - 4 * j
                    off = 128 * d if (causal and d >= 0) else 0
                    p2 = p2s[t]
                    if causal:
                        if d >= 0:
                            for i in range(2):
                                nc.vector.tensor_mul(
                                    p2[:, i, off : off + 128],
                                    p2[:, i, off : off + 128],
                                    mask128[:],
                                )
                    else:
                        mt_t = mpool.tile([128, NQ], bf16, tag="mt")
                        nc.sync.dma_start(
                            mt_t[:],
                            maskT[128 * t : 128 * t + 128, NQ * j : NQ * j + NQ],
                        )
                        for i in range(2):
                            nc.vector.tensor_mul(p2[:, i, :], p2[:, i, :], mt_t[:])
                    for i in range(2):
                        g = 2 * c2 + i
                        lo = 64 - 64 * i  # i=0 -> [64:192], i=1 -> [0:128]
                        nc.tensor.matmul(
                            avt[:, i, off:NQ],
                            vproj[:, t, g, lo : lo + 128],
                            p2[:, i, off:NQ],
                            start=(t == 0),
                            stop=(t == ktiles - 1),
                        )

                def unit(t):
                    sc_exp(t)
                    mask_av(t)

                return ktiles, sc_exp, mask_av, unit

            def normalize_evac(avt, ring):
                # One copy releases the av PSUM banks; then the denominator
                # rows (row 64 chunk0 / row 0 chunk1) are spread over all 128
                # lanes via DMA reshapes (reciprocal is ~6.4ns/elem/lane).
                # The remaining stages run as fillers of LATER pairs, each
                # emitted only once its dependency has had time to complete
                # -- chain ops must never head-of-line-block an engine queue
                # that critical ops (mask muls) share.
                src = rpool.tile([128, 2, NQ], f32, tag="avs", bufs=3)
                nc.vector.tensor_copy(src[:], avt[:])
                rq = rpool.tile([128, 8], f32, tag="rq", bufs=3)
                ring.dma_start(rq[:, 0:4], src[64:65, 0, :])
                ring.dma_start(rq[:, 4:8], src[0:1, 1, :])
                return src, rq

            def chain_b(src_rq, ring):
                # stage B: lane-parallel reciprocal + return DMAs
                src, rq = src_rq
                rqr = rpool.tile([128, 8], f32, tag="rqr")
                nc.vector.reciprocal(rqr[:], rq[:])
                rz = rpool.tile([1, 2, NQ], f32, tag="rz")
                ring.dma_start(rz[0:1, 0, :], rqr[:, 0:4])
                ring.dma_start(rz[0:1, 1, :], rqr[:, 4:8])
                return rz

            def chain_c1(rz):
                # stage C1: broadcast across partitions on the idle gpsimd
                rb = rpool.tile([128, 2, NQ], f32, tag="rb")
                nc.gpsimd.partition_broadcast(rb[:, 1, :], rz[0:1, 1, :],
                                              channels=128)
                nc.gpsimd.partition_broadcast(rb[0:64, 0, :], rz[0:1, 0, :],
                                              channels=64)
                return rb

            def chain_c2(c2, j, src_rq, rb):
                # stage C2: final multiplies on DVE, emitted a few filler
                # slots after C1 so the pbcasts are done by then
                src, _ = src_rq
                nc.vector.tensor_mul(
                    attn2[64:128, c2, NQ * j : NQ * j + NQ],
                    src[64:128, 1, :],
                    rb[64:128, 1, :],
                )
                nc.vector.tensor_mul(
                    attn2[0:64, c2, NQ * j : NQ * j + NQ],
                    src[0:64, 0, :],
                    rb[0:64, 0, :],
                )

            def normalize_tail(c2, j, avt):
                # Tail variant, no DMA and no full evacuation: copy just the
                # two PE-replicated denominator halves (chunk-swapped) into
                # one [128, NQ] tile, ONE lane-parallel reciprocal covers
                # both heads, then pbcast (1/den1) + stream_shuffle quadrant
                # moves (1/den0) land them on the av rows; the multiplies
                # read av straight out of PSUM (nobody needs the banks).
                dcomb = rpool.tile([128, NQ], f32, tag="dcomb", bufs=1)
                nc.vector.tensor_copy(dcomb[0:64, :], avt[0:64, 1, :])
                nc.vector.tensor_copy(dcomb[64:128, :], avt[64:128, 0, :])
                rr = rpool.tile([128, NQ], f32, tag="rr", bufs=1)
                nc.vector.reciprocal(rr[:], dcomb[:])
                rbt = rpool.tile([128, NQ], f32, tag="trb", bufs=1)
                nc.gpsimd.partition_broadcast(rbt[:, :], rr[0:1, :],
                                              channels=128)
                tsh = rpool.tile([64, NQ], f32, tag="tsh", bufs=1)
                ident = list(range(32))
                nc.vector.stream_shuffle(tsh[0:32, :], rr[64:96, :], ident)
                nc.vector.stream_shuffle(tsh[32:64, :], rr[96:128, :], ident)
                nc.vector.tensor_mul(
                    attn2[0:64, c2, NQ * j : NQ * j + NQ],
                    avt[0:64, 0, :],
                    tsh[0:64, :],
                )
                nc.vector.tensor_mul(
                    attn2[64:128, c2, NQ * j : NQ * j + NQ],
                    avt[64:128, 1, :],
                    rbt[64:128, :],
                )

            def attn_pair(c2, j, fillers=(), spread_ring=None, tail=False):
                """Emit one head-pair x q-tile attention, interleaving filler
                emitters (projection groups, oproj chunks, deferred normalize
                chain stages) between kv-tile units.  Returns (b, c1, c2)
                stage emitters for the deferred normalize; the evacuation +
                denominator-spread DMAs are emitted immediately so the PSUM
                banks recycle and the transfers complete during later
                pairs."""
                avt = psum.tile([128, 2, NQ], f32, tag="av", bufs=1,
                                name=f"av{c2}{j}")
                ktiles, sc_exp, mask_av, unit = attn_units(c2, j, avt)

                def spread(emitters, fill):
                    nf, nu = len(fill), len(emitters)
                    fi = 0
                    for ui, u in enumerate(emitters):
                        u()
                        while fi < nf and fi * nu <= (ui + 1) * nf - 1:
                            fill[fi]()
                            fi += 1
                    while fi < nf:
                        fill[fi]()
                        fi += 1

                spread([lambda t=t: unit(t) for t in range(ktiles)],
                       list(fillers))
                if tail:
                    normalize_tail(c2, j, avt)
                    return None, None, None
                src_rq = normalize_evac(avt, spread_ring)

                state = {}

                def b(ring):
                    def f():
                        state["rz"] = chain_b(src_rq, ring)
                    return f

                def stage_c1():
                    state["rb"] = chain_c1(state["rz"])

                def stage_c2():
                    chain_c2(c2, j, src_rq, state["rb"])

                return b, stage_c1, stage_c2

            ostage = {}

            def oproj_m(n, m, act_copy=False, staged=True):
                ps = psum.tile([128, NQ], f32, tag="mm", bufs=2)
                for c2 in range(2):
                    nc.tensor.matmul(
                        ps[:],
                        wo2[:, c2, 128 * m : 128 * m + 128],
                        attn2[:, c2, NQ * n : NQ * n + NQ],
                        start=(c2 == 0),
                        stop=(c2 == 1),
                    )
                if staged:
                    # stage all 8 m-blocks, then one 1MB out-DMA per n keeps
                    # the sync queue at 4 big transfers instead of 32 small
                    if n not in ostage:
                        ostage[n] = opool.tile([128, KT, NQ], bf16, tag="ot",
                                               bufs=2, name=f"ot{n}")
                    ot = ostage[n]
                    if act_copy:
                        nc.scalar.copy(ot[:, m, :], ps[:])
                    else:
                        nc.vector.tensor_copy(ot[:, m, :], ps[:])
                    if m == KT - 1:
                        nc.sync.dma_start(
                            out[:, NQ * n : NQ * n + NQ].rearrange(
                                "(m p) q -> p m q", p=128
                            ),
                            ot[:],
                        )
                else:
                    ot = opool.tile([128, NQ], bf16, tag="otm", bufs=4)
                    if act_copy:
                        nc.scalar.copy(ot[:], ps[:])
                    else:
                        nc.vector.tensor_copy(ot[:], ps[:])
                    nc.sync.dma_start(
                        out[128 * m : 128 * m + 128, NQ * n : NQ * n + NQ],
                        ot[:],
                    )

            def oproj_n(n, alternate=False, staged=True):
                # alternate=True splits the PSUM->SBUF copies between DVE and
                # ACT -- used for the tail block where ACT has no exp left.
                for m in range(D // 128):
                    oproj_m(n, m, act_copy=alternate and (m % 2 == 1),
                            staged=staged)

            def Q(m2, n):
                return lambda: qkproj("q", m2, n)

            def K_(m2, n):
                return lambda: qkproj("k", m2, n)

            def V2(n, half):
                mts = range(4 * n + 2 * half, 4 * n + 2 * half + 2)
                def f():
                    for mt in mts:
                        vproj_tile(mt)
                return f

            def OP(n, ms):
                def f():
                    for m in ms:
                        # alternate PSUM->SBUF copies between DVE and ACT
                        oproj_m(n, m, act_copy=(m % 2 == 1))
                return f

            # ---- emission schedule ----
            # PE is the binding engine: keep it dense.  Fillers are spread
            # between attention units; a pair's fillers must not be among its
            # own dependencies EXCEPT vproj fillers, which land earlier in the
            # unit loop than the first unit that reads them (verified against
            # the spread formula).  Deferred normalize chains run as fillers
            # of the following pair, on the scalar ring while the sync ring
            # still streams inputs (~first 55us), sync after.  Extra warmup
            # blocks cover the k0/v0 DMA arrival gaps.  The small (1,0) pair
            # runs last to keep the post-exp tail short.
            # Early chains (pairs ending before the ~60us bulk-input drain)
            # have their B stage deferred TWO pairs so the spread DMAs never
            # head-of-line-block the DVE queue; later chains use one-pair
            # staging (DMA latency drops once the bulk transfers finish).
            Q(0, 0)()
            Q(1, 0)()
            warmup(16)
            K_(0, 0)()
            K_(1, 0)()
            warmup(12)
            V2(0, 0)()
            V2(0, 1)()
            # Chain DMA latency is 10-20us while bulk inputs stream
            # (<60us), so the first two chains defer their B stage two pairs;
            # later chains use one-pair staging.  Stages are placed in the
            # mask-free first-4j-unit windows of their host pair so they
            # never head-of-line-block the DVE queue ahead of mask muls.
            b00, c00a, c00b = attn_pair(0, 0, fillers=[Q(0, 1), Q(1, 1),
                                                       K_(0, 1), K_(1, 1)],
                                        spread_ring=nc.scalar)
            b10, c10a, c10b = attn_pair(1, 0, fillers=[Q(1, 2)],
                                        spread_ring=nc.scalar)
            b01, c01a, c01b = attn_pair(0, 1, fillers=[V2(1, 0), V2(1, 1),
                                                       Q(0, 2), K_(0, 2)],
                                        spread_ring=nc.scalar)
            b11, c11a, c11b = attn_pair(1, 1, fillers=[b00(nc.scalar),
                                                       Q(0, 3), c00a,
                                                       K_(1, 2), c00b,
                                                       b10(nc.scalar)],
                                        spread_ring=nc.scalar)
            b02, c02a, c02b = attn_pair(0, 2, fillers=[c10a, V2(2, 0), c10b,
                                                       b01(nc.scalar),
                                                       V2(2, 1), c01a,
                                                       K_(0, 3), c01b,
                                                       b11(nc.scalar)],
                                        spread_ring=nc.sync)
            b12, c12a, c12b = attn_pair(1, 2, fillers=[c11a, Q(1, 3), c11b,
                                                       K_(1, 3), b02(nc.sync),
                                                       OP(0, range(0, 4)),
                                                       c02a,
                                                       OP(0, range(4, 8)),
                                                       c02b],
                                        spread_ring=nc.sync)
            b03, c03a, c03b = attn_pair(0, 3, fillers=[b12(nc.sync),
                                                       V2(3, 0), c12a,
                                                       V2(3, 1), c12b,
                                                       OP(1, range(0, 4)),
                                                       OP(1, range(4, 8))],
                                        spread_ring=nc.sync)
            attn_pair(1, 3, fillers=[b03(nc.sync),
                                     OP(2, range(0, 3)),
                                     c03a,
                                     OP(2, range(3, 6)),
                                     c03b,
                                     OP(2, range(6, 8))],
                      tail=True)
            oproj_n(3, alternate=True, staged=False)

    nc.compile()
    return nc


def _get_program(causal: bool):
    if causal not in _programs:
        _programs[causal] = _build_program(causal)
    return _programs[causal]


def kernel(query, key, value, mask, Wq, Wk, Wv, Wo):
    global last_results
    from concourse.bass_utils import run_bass_kernel_spmd

    query = np.asarray(query, dtype=np.float32)
    key = np.asarray(key, dtype=np.float32)
    value = np.asarray(value, dtype=np.float32)
    Wq = np.asarray(Wq, dtype=np.float32)
    Wk = np.asarray(Wk, dtype=np.float32)
    Wv = np.asarray(Wv, dtype=np.float32)
    Wo = np.asarray(Wo, dtype=np.float32)
    m2d = np.asarray(mask).reshape(S, S).astype(bool)

    causal = bool(np.array_equal(m2d, np.tril(np.ones((S, S), dtype=bool))))
    nc = _get_program(causal)

    scale = 1.0 / math.sqrt(DH)
    WqT = np.ascontiguousarray((Wq * scale).T).astype(_BF16)
    WkT = np.ascontiguousarray(Wk.T).astype(_BF16)
    WvT = np.ascontiguousarray(Wv.T).astype(_BF16)
    WoT = np.ascontiguousarray(Wo.T).astype(_BF16)

    def tile_x(xTb):
        # [D, S] -> [QTILES, 128, KT*512]: group n holds token-columns
        # [512n, 512n+512) of all KT row-tiles, 8KB contiguous per partition
        return np.ascontiguousarray(
            xTb.reshape(KT, 128, QTILES, NQ).transpose(2, 1, 0, 3).reshape(
                QTILES, 128, KT * NQ
            )
        )

    def tile_w(wT):
        # [D, 256] -> [128, KT*256]
        return np.ascontiguousarray(
            wT.reshape(KT, 128, 256).transpose(1, 0, 2).reshape(128, KT * 256)
        )

    def tile_wo(woTs):
        # [256, D] -> [128, 2*D]: head h rows at 64*(h%2), chunk h//2
        o = np.zeros((128, 2, D), dtype=woTs.dtype)
        for h in range(GRP):
            base = 64 * (h % 2)
            o[base : base + 64, h // 2, :] = woTs[64 * h : 64 * h + 64, :]
        return np.ascontiguousarray(o.reshape(128, 2 * D))

    xT = {
        "qT": [tile_x(query[b].T.astype(_BF16)) for b in range(B)],
        "kT": [tile_x(key[b].T.astype(_BF16)) for b in range(B)],
        "vT": [tile_x(value[b].T.astype(_BF16)) for b in range(B)],
    }
    if not causal:
        maskT = np.ascontiguousarray(m2d.T).astype(_BF16)

    in_maps = []
    for c in range(NCORES):
        b, g = c // 4, c % 4
        sl = slice(256 * g, 256 * g + 256)
        im = {
            "qT": xT["qT"][b],
            "kT": xT["kT"][b],
            "vT": xT["vT"][b],
            "wqT": tile_w(WqT[:, sl]),
            "wkT": tile_w(WkT[:, sl]),
            "wvT": tile_w(WvT[:, sl]),
            "woT": tile_wo(WoT[sl, :]),
        }
        if not causal:
            im["maskT"] = maskT
        in_maps.append(im)

    trace = os.environ.get("KERNEL_PROFILE", "") == "1"
    res = run_bass_kernel_spmd(nc, in_maps, list(range(NCORES)), trace=trace)
    last_results = res

    outp = np.empty((B, S, D), dtype=np.float32)
    for b in range(B):
        acc = res.results[4 * b]["out"].astype(np.float32)
        for g in range(1, 4):
            acc = acc + res.results[4 * b + g]["out"].astype(np.float32)
        outp[b] = acc.T
    return outp


# revision 38
# speedup vs baseline: 1.1230x; 1.1230x over previous
"""Multi-head attention (B=2, S=2048, D=1024, H=16, causal) on 8 TRN2 NeuronCores.

Sharding: 8 cores = 2 batches x 4 head-groups (4 heads each).  Each core
computes the QKV projections for its head slice, causal attention for its 4
heads, and the partial output projection (input-dim slice of Wo).  The
all-reduce over head-groups happens at gather time on the host (sum of 4
partials per batch).

Everything on device works in token-transposed layout ([feature, token]):
  scores^T[kv, q] = K_projT_tile^T @ Q_projT   (K = dh = 64)
  P = exp(scores^T)  (no max subtraction: scores ~ N(0,1), |s| < ~7)
  out^T = [ones|V|ones] slices^T @ P           (ones half -> softmax denom,
                                                replicated across 64 rows)
  partial^T[dmodel, tok] = WoT_slice^T @ attn_out^T

v3 restructure (from trace analysis of the 184.7us v2):
  - softmax denominator replicated across 64 PSUM partitions by the PE
    itself: AV stationary is a 192-wide [ones|V|ones] band, sliced
    [64:192] for even heads (av rows 0..63, denom rows 64..127) and
    [0:128] for odd heads (denom rows 0..63, av rows 64..127).  M=128 vs
    65 costs zero extra PE cycles.  normalize is then: one PSUM->SBUF
    evacuation copy, two reciprocal_approx_fast, two 64-partition shift
    DMAs on the idle sync ring, two aligned DVE multiplies straight into
    attn2.  No gpsimd partition_broadcast / DMA round trips (v2's 10.6us
    tail stall).
  - q/k/v projection PSUM->SBUF copies moved from DVE to the scalar
    engine (Copy lives in every ACT table set; ACT is idle during the
    projection phase).  DVE keeps mask muls, oproj copies, normalize.
  - startup: tiny warm memset is the FIRST gpsimd op (warmup matmuls
    start ~1.5us, was 12.5us); wk/k0 DMAs move to the scalar ring;
    vproj ones-memsets run on DVE.
"""

import math
import os

import numpy as np
import ml_dtypes

_BF16 = ml_dtypes.bfloat16

B, S, D = 2, 2048, 1024
H, DH = 16, 64
NCORES = 8
GRP = 4  # heads per core
KT = D // 128  # 8 k-tiles over d_model
NQ = 512  # q tile width
QTILES = S // NQ  # 4
KVTILES = S // 128  # 16

last_results = None

_programs = {}


def _build_program(causal: bool):
    import concourse.bass as bass
    import concourse.mybir as mybir
    import concourse.tile as tile
    from concourse import bacc

    f32 = mybir.dt.float32
    bf16 = mybir.dt.bfloat16
    Exp = mybir.ActivationFunctionType.Exp

    nc = bacc.Bacc(
        "TRN2",
        target_bir_lowering=False,
        debug=False,
        enable_asserts=False,
        num_devices=NCORES,
    )

    # all inputs host-pre-tiled so every DMA is one instruction with >=8KB
    # contiguous per-partition lines (1KB lines are packet-bound at ~97GB/s)
    qT = nc.dram_tensor("qT", [QTILES, 128, KT * NQ], bf16, kind="ExternalInput").ap()
    kT = nc.dram_tensor("kT", [QTILES, 128, KT * NQ], bf16, kind="ExternalInput").ap()
    vT = nc.dram_tensor("vT", [QTILES, 128, KT * NQ], bf16, kind="ExternalInput").ap()
    wqT = nc.dram_tensor("wqT", [128, KT * 256], bf16, kind="ExternalInput").ap()
    wkT = nc.dram_tensor("wkT", [128, KT * 256], bf16, kind="ExternalInput").ap()
    wvT = nc.dram_tensor("wvT", [128, KT * 256], bf16, kind="ExternalInput").ap()
    woT = nc.dram_tensor("woT", [128, 2 * D], bf16, kind="ExternalInput").ap()
    if not causal:
        maskT = nc.dram_tensor("maskT", [S, S], bf16, kind="ExternalInput").ap()
    out = nc.dram_tensor("out", [D, S], bf16, kind="ExternalOutput").ap()

    with tile.TileContext(nc) as tc:
        with (
            tc.tile_pool(name="persist", bufs=1) as sb,
            tc.tile_pool(name="stream", bufs=3) as stream,
            tc.tile_pool(name="psum", bufs=1, space="PSUM") as psum,
            tc.tile_pool(name="p_sb", bufs=6) as pbuf,
            tc.tile_pool(name="r_sb", bufs=2) as rpool,
            tc.tile_pool(name="m_sb", bufs=4) as mpool,
            tc.tile_pool(name="o_sb", bufs=4) as opool,
        ):
            # ---- persistent SBUF tensors ----
            wq_sb = sb.tile([128, KT, 256], bf16)
            wk_sb = sb.tile([128, KT, 256], bf16)
            wv_sb = sb.tile([128, KT, 256], bf16)
            wo2 = sb.tile([128, 2, D], bf16)  # head h at rows 64*(h%2), chunk h//2
            qproj = sb.tile([128, 2, S], bf16)
            kproj = sb.tile([128, 2, S], bf16)
            attn2 = sb.tile([128, 2, S], bf16)  # head h at rows 64*(h%2), chunk h//2

            # V with 64-wide ones flanks: per (kv-tile, head g): cols 0..63 =
            # ones, 64..127 = V, 128..191 = ones.  Stationary [64:192] ->
            # av rows 0..63 + denom replicated on rows 64..127 (even heads);
            # [0:128] -> denom replicated rows 0..63 + av rows 64..127 (odd
            # heads).  Replication is free on the PE (M=128 vs 65 costs no
            # cycles) and lets the tail chain run a direct lane-parallel
            # reciprocal with no DMA round trip.
            vproj = sb.tile([128, KVTILES, GRP, 192], bf16)

            # ---- gpsimd queue: tiny warm memset FIRST so warmup matmuls
            # are unblocked right after the engine preamble (~5us).
            warm = sb.tile([128, 256], bf16)
            nc.gpsimd.memset(warm[:], 0.0)

            # PE warmup: dummy matmuls fill the DMA-paced startup window and
            # keep the clock ramp going until real matmuls start.
            _warm_ctr = [0]

            def warmup(k):
                for _ in range(k):
                    w = _warm_ctr[0]
                    _warm_ctr[0] += 1
                    wp = psum.tile(
                        [128, 256], f32, tag="mm", bufs=2, name=f"warm{w}"
                    )
                    nc.tensor.matmul(
                        wp[:], warm[:, 0:128], warm[:], start=True, stop=True
                    )

            # ---- input DMA prologue.  Aggregate DMA bandwidth is ~280GB/s
            # shared across the three rings (sync HWDGE, scalar HWDGE,
            # gpsimd SWDGE), so what matters is global need-order: wq+q0
            # first (first projection at ~12us), then k0+wk, wv+v0, then the
            # later groups.  Each 1MB token group is split into two 0.5MB
            # halves on different rings to keep arrival granularity fine.
            qg, kg, vg = [], [], []

            def _mktile(dst_list, tag):
                n = len(dst_list)
                t = stream.tile(
                    [128, KT, NQ], bf16, tag=tag, bufs=3, name=f"{tag}{n}"
                )
                dst_list.append(t)
                return t

            def _half(t, src, n, half, eng):
                h = KT // 2
                sl = slice(half * h, (half + 1) * h)
                eng.dma_start(t[:, sl, :], src[n][:, half * h * NQ : (half + 1) * h * NQ])

            # global need order: wq, q0 | wk, k0 | v0, q1 | k1, v1 | q2, k2 |
            # q3, k3 | v2, v3 (wo anywhere past ~30us).  q0/k0 are halved
            # across sync+gpsimd so the first projections start ~12us; the
            # rest are full 1MB transfers (fewer completion bubbles on the 8
            # DMA semaphore lanes).  Ring FIFO order matches need order.
            nc.scalar.dma_start(wq_sb[:], wqT[:])
            t = _mktile(qg, "qg")
            _half(t, qT, 0, 0, nc.sync)
            _half(t, qT, 0, 1, nc.gpsimd)
            t = _mktile(kg, "kg")
            _half(t, kT, 0, 0, nc.sync)
            _half(t, kT, 0, 1, nc.gpsimd)
            nc.scalar.dma_start(wk_sb[:], wkT[:])
            nc.scalar.dma_start(wv_sb[:], wvT[:])
            t = _mktile(vg, "vg")
            nc.sync.dma_start(t[:], vT[0])
            t = _mktile(qg, "qg")
            nc.gpsimd.dma_start(t[:], qT[1])
            t = _mktile(kg, "kg")
            nc.sync.dma_start(t[:], kT[1])
            t = _mktile(vg, "vg")
            nc.sync.dma_start(t[:], vT[1])
            t = _mktile(qg, "qg")
            nc.gpsimd.dma_start(t[:], qT[2])
            t = _mktile(kg, "kg")
            nc.gpsimd.dma_start(t[:], kT[2])
            nc.scalar.dma_start(wo2[:], woT[:])
            t = _mktile(qg, "qg")
            nc.sync.dma_start(t[:], qT[3])
            t = _mktile(kg, "kg")
            nc.gpsimd.dma_start(t[:], kT[3])
            t = _mktile(vg, "vg")
            nc.gpsimd.dma_start(t[:], vT[2])
            t = _mktile(vg, "vg")
            nc.sync.dma_start(t[:], vT[3])

            # gpsimd setup after its DMA issues: causal mask (needed ~28us)
            if causal:
                # single 128x128 causal block: keep where q_local >= kv_local
                mask128 = sb.tile([128, 128], bf16)
                nc.gpsimd.memset(mask128[:], 1.0)
                nc.gpsimd.affine_select(
                    out=mask128[:],
                    in_=mask128[:],
                    compare_op=mybir.AluOpType.is_ge,
                    fill=0.0,
                    base=0,
                    pattern=[[1, 128]],
                    channel_multiplier=-1,
                )
            # ones flanks on gpsimd (its queue is idle once the DMA
            # issues are out; done by ~20us, first AV needs them ~28us)
            nc.gpsimd.memset(vproj[:, :, :, 0:64], 1.0)
            nc.gpsimd.memset(vproj[:, :, :, 128:192], 1.0)

            warmup(24)

            def qkproj(which, m2, n):
                w_sb = wq_sb if which == "q" else wk_sb
                xt = (qg if which == "q" else kg)[n]
                proj = qproj if which == "q" else kproj
                ps = psum.tile([128, NQ], f32, tag="mm", bufs=2)
                for kt in range(KT):
                    nc.tensor.matmul(
                        ps[:],
                        w_sb[:, kt, 128 * m2 : 128 * m2 + 128],
                        xt[:, kt, :],
                        start=(kt == 0),
                        stop=(kt == KT - 1),
                    )
                nc.scalar.copy(proj[:, m2, NQ * n : NQ * n + NQ], ps[:])

            def vproj_tile(mt):
                vt = vg[mt // 4]
                col = 128 * (mt % 4)
                ps = psum.tile([128, 256], f32, tag="mm", bufs=2)
                for kt in range(KT):
                    nc.tensor.matmul(
                        ps[:],
                        vt[:, kt, col : col + 128],
                        wv_sb[:, kt, :],
                        start=(kt == 0),
                        stop=(kt == KT - 1),
                    )
                nc.scalar.copy(
                    vproj[:, mt, :, 64:128],
                    ps[:].rearrange("p (h d) -> p h d", h=GRP),
                )

            # attention state per (c2): av psum tile + per-tile units
            def attn_units(c2, j, avt):
                """Yield per-kv-tile unit emitters for head pair c2, q-tile j."""
                ktiles = 4 * j + 4 if causal else KVTILES
                p2s = {}

                def sc_exp(t):
                    d = t - 4 * j
                    off = 128 * d if (causal and d >= 0) else 0
                    sc = psum.tile([128, 2, NQ], f32, tag="sc", bufs=2)
                    for i in range(2):
                        base = 64 * i
                        nc.tensor.matmul(
                            sc[:, i, off:NQ],
                            kproj[base : base + 64, c2, 128 * t : 128 * t + 128],
                            qproj[base : base + 64, c2, NQ * j + off : NQ * j + NQ],
                            start=True,
                            stop=True,
                        )
                    p2 = pbuf.tile([128, 2, NQ], bf16, tag="p")
                    p2s[t] = p2
                    nc.scalar.activation(p2[:, :, off:NQ], sc[:, :, off:NQ], Exp)

                def mask_av(t):
                    d = t - 4 * j
                    off = 128 * d if (causal and d >= 0) else 0
                    p2 = p2s[t]
                    if causal:
                        if d >= 0:
                            for i in range(2):
                                nc.vector.tensor_mul(
                                    p2[:, i, off : off + 128],
                                    p2[:, i, off : off + 128],
                                    mask128[:],
                                )
                    else:
                        mt_t = mpool.tile([128, NQ], bf16, tag="mt")
                        nc.sync.dma_start(
                            mt_t[:],
                            maskT[128 * t : 128 * t + 128, NQ * j : NQ * j + NQ],
                        )
                        for i in range(2):
                            nc.vector.tensor_mul(p2[:, i, :], p2[:, i, :], mt_t[:])
                    for i in range(2):
                        g = 2 * c2 + i
                        lo = 64 - 64 * i  # i=0 -> [64:192], i=1 -> [0:128]
                        nc.tensor.matmul(
                            avt[:, i, off:NQ],
                            vproj[:, t, g, lo : lo + 128],
                            p2[:, i, off:NQ],
                            start=(t == 0),
                            stop=(t == ktiles - 1),
                        )

                def unit(t):
                    sc_exp(t)
                    mask_av(t)

                return ktiles, sc_exp, mask_av, unit

            def normalize_evac(avt, ring):
                # One copy releases the av PSUM banks; then the denominator
                # rows (row 64 chunk0 / row 0 chunk1) are spread over all 128
                # lanes via DMA reshapes (reciprocal is ~6.4ns/elem/lane).
                # The remaining stages run as fillers of LATER pairs, each
                # emitted only once its dependency has had time to complete
                # -- chain ops must never head-of-line-block an engine queue
                # that critical ops (mask muls) share.
                src = rpool.tile([128, 2, NQ], f32, tag="avs", bufs=3)
                nc.vector.tensor_copy(src[:], avt[:])
                rq = rpool.tile([128, 8], f32, tag="rq", bufs=3)
                ring.dma_start(rq[:, 0:4], src[64:65, 0, :])
                ring.dma_start(rq[:, 4:8], src[0:1, 1, :])
                return src, rq

            def chain_b(src_rq, ring):
                # stage B: lane-parallel reciprocal + return DMAs
                src, rq = src_rq
                rqr = rpool.tile([128, 8], f32, tag="rqr")
                nc.vector.reciprocal(rqr[:], rq[:])
                rz = rpool.tile([1, 2, NQ], f32, tag="rz")
                ring.dma_start(rz[0:1, 0, :], rqr[:, 0:4])
                ring.dma_start(rz[0:1, 1, :], rqr[:, 4:8])
                return rz

            def chain_c1(rz):
                # stage C1: broadcast across partitions on the idle gpsimd
                rb = rpool.tile([128, 2, NQ], f32, tag="rb")
                nc.gpsimd.partition_broadcast(rb[:, 1, :], rz[0:1, 1, :],
                                              channels=128)
                nc.gpsimd.partition_broadcast(rb[0:64, 0, :], rz[0:1, 0, :],
                                              channels=64)
                return rb

            def chain_c2(c2, j, src_rq, rb):
                # stage C2: final multiplies on DVE, emitted a few filler
                # slots after C1 so the pbcasts are done by then
                src, _ = src_rq
                nc.vector.tensor_mul(
                    attn2[64:128, c2, NQ * j : NQ * j + NQ],
                    src[64:128, 1, :],
                    rb[64:128, 1, :],
                )
                nc.vector.tensor_mul(
                    attn2[0:64, c2, NQ * j : NQ * j + NQ],
                    src[0:64, 0, :],
                    rb[0:64, 0, :],
                )

            def normalize_tail(c2, j, avt):
                # Tail variant, no DMA and no full evacuation: copy just the
                # two PE-replicated denominator halves (chunk-swapped) into
                # one [128, NQ] tile, ONE lane-parallel reciprocal covers
                # both heads, then pbcast (1/den1) + stream_shuffle quadrant
                # moves (1/den0) land them on the av rows; the multiplies
                # read av straight out of PSUM (nobody needs the banks).
                dcomb = rpool.tile([128, NQ], f32, tag="dcomb", bufs=1)
                nc.vector.tensor_copy(dcomb[0:64, :], avt[0:64, 1, :])
                nc.vector.tensor_copy(dcomb[64:128, :], avt[64:128, 0, :])
                rr = rpool.tile([128, NQ], f32, tag="rr", bufs=1)
                nc.vector.reciprocal(rr[:], dcomb[:])
                rbt = rpool.tile([128, NQ], f32, tag="trb", bufs=1)
                nc.gpsimd.partition_broadcast(rbt[:, :], rr[0:1, :],
                                              channels=128)
                tsh = rpool.tile([64, NQ], f32, tag="tsh", bufs=1)
                ident = list(range(32))
                nc.vector.stream_shuffle(tsh[0:32, :], rr[64:96, :], ident)
                nc.vector.stream_shuffle(tsh[32:64, :], rr[96:128, :], ident)
                nc.vector.tensor_mul(
                    attn2[0:64, c2, NQ * j : NQ * j + NQ],
                    avt[0:64, 0, :],
                    tsh[0:64, :],
                )
                nc.vector.tensor_mul(
                    attn2[64:128, c2, NQ * j : NQ * j + NQ],
                    avt[64:128, 1, :],
                    rbt[64:128, :],
                )

            def attn_pair(c2, j, fillers=(), spread_ring=None, tail=False):
                """Emit one head-pair x q-tile attention, interleaving filler
                emitters (projection groups, oproj chunks, deferred normalize
                chain stages) between kv-tile units.  Returns (b, c1, c2)
                stage emitters for the deferred normalize; the evacuation +
                denominator-spread DMAs are emitted immediately so the PSUM
                banks recycle and the transfers complete during later
                pairs."""
                avt = psum.tile([128, 2, NQ], f32, tag="av", bufs=1,
                                name=f"av{c2}{j}")
                ktiles, sc_exp, mask_av, unit = attn_units(c2, j, avt)

                def spread(emitters, fill):
                    nf, nu = len(fill), len(emitters)
                    fi = 0
                    for ui, u in enumerate(emitters):
                        u()
                        while fi < nf and fi * nu <= (ui + 1) * nf - 1:
                            fill[fi]()
                            fi += 1
                    while fi < nf:
                        fill[fi]()
                        fi += 1

                spread([lambda t=t: unit(t) for t in range(ktiles)],
                       list(fillers))
                if tail:
                    normalize_tail(c2, j, avt)
                    return None, None, None
                src_rq = normalize_evac(avt, spread_ring)

                state = {}

                def b(ring):
                    def f():
                        state["rz"] = chain_b(src_rq, ring)
                    return f

                def stage_c1():
                    state["rb"] = chain_c1(state["rz"])

                def stage_c2():
                    chain_c2(c2, j, src_rq, state["rb"])

                return b, stage_c1, stage_c2

            ostage = {}

            def oproj_m(n, m, act_copy=False, staged=True):
                ps = psum.tile([128, NQ], f32, tag="mm", bufs=2)
                for c2 in range(2):
                    nc.tensor.matmul(
                        ps[:],
                        wo2[:, c2, 128 * m : 128 * m + 128],
                        attn2[:, c2, NQ * n : NQ * n + NQ],
                        start=(c2 == 0),
                        stop=(c2 == 1),
                    )
                # per-m out DMAs: a staged 1MB [p m q] DMA has 1KB dram lines
                # (packet-bound ~97GB/s => ~11us) and ring-order-blocks later
                # sync DMAs (tail chain spreads, final outs) -- keep 128KB
                # transfers that pipeline behind each copy instead
                ot = opool.tile([128, NQ], bf16, tag="otm", bufs=4)
                if act_copy:
                    nc.scalar.copy(ot[:], ps[:])
                else:
                    nc.vector.tensor_copy(ot[:], ps[:])
                nc.sync.dma_start(
                    out[128 * m : 128 * m + 128, NQ * n : NQ * n + NQ],
                    ot[:],
                )

            def oproj_n(n, alternate=False, staged=True):
                # alternate=True splits the PSUM->SBUF copies between DVE and
                # ACT -- used for the tail block where ACT has no exp left.
                for m in range(D // 128):
                    oproj_m(n, m, act_copy=alternate and (m % 2 == 1),
                            staged=staged)

            def Q(m2, n):
                return lambda: qkproj("q", m2, n)

            def K_(m2, n):
                return lambda: qkproj("k", m2, n)

            def V2(n, half):
                mts = range(4 * n + 2 * half, 4 * n + 2 * half + 2)
                def f():
                    for mt in mts:
                        vproj_tile(mt)
                return f

            def OP(n, ms):
                def f():
                    for m in ms:
                        # alternate PSUM->SBUF copies between DVE and ACT
                        oproj_m(n, m, act_copy=(m % 2 == 1))
                return f

            # ---- emission schedule ----
            # PE is the binding engine: keep it dense.  Fillers are spread
            # between attention units; a pair's fillers must not be among its
            # own dependencies EXCEPT vproj fillers, which land earlier in the
            # unit loop than the first unit that reads them (verified against
            # the spread formula).  Deferred normalize chains run as fillers
            # of the following pair, on the scalar ring while the sync ring
            # still streams inputs (~first 55us), sync after.  Extra warmup
            # blocks cover the k0/v0 DMA arrival gaps.  The small (1,0) pair
            # runs last to keep the post-exp tail short.
            # Early chains (pairs ending before the ~60us bulk-input drain)
            # have their B stage deferred TWO pairs so the spread DMAs never
            # head-of-line-block the DVE queue; later chains use one-pair
            # staging (DMA latency drops once the bulk transfers finish).
            Q(0, 0)()
            Q(1, 0)()
            warmup(16)
            K_(0, 0)()
            K_(1, 0)()
            warmup(12)
            V2(0, 0)()
            V2(0, 1)()
            # Chain DMA latency is 10-20us while bulk inputs stream
            # (<60us), so the first two chains defer their B stage two pairs;
            # later chains use one-pair staging.  Stages are placed in the
            # mask-free first-4j-unit windows of their host pair so they
            # never head-of-line-block the DVE queue ahead of mask muls.
            b00, c00a, c00b = attn_pair(0, 0, fillers=[Q(0, 1), Q(1, 1),
                                                       K_(0, 1), K_(1, 1)],
                                        spread_ring=nc.scalar)
            b10, c10a, c10b = attn_pair(1, 0, fillers=[Q(1, 2)],
                                        spread_ring=nc.scalar)
            b01, c01a, c01b = attn_pair(0, 1, fillers=[V2(1, 0), V2(1, 1),
                                                       Q(0, 2), K_(0, 2)],
                                        spread_ring=nc.scalar)
            b11, c11a, c11b = attn_pair(1, 1, fillers=[b00(nc.scalar),
                                                       Q(0, 3), c00a,
                                                       K_(1, 2), c00b,
                                                       b10(nc.scalar)],
                                        spread_ring=nc.scalar)
            b02, c02a, c02b = attn_pair(0, 2, fillers=[c10a, V2(2, 0), c10b,
                                                       b01(nc.scalar),
                                                       V2(2, 1), c01a,
                                                       K_(0, 3), c01b,
                                                       b11(nc.scalar)],
                                        spread_ring=nc.sync)
            b12, c12a, c12b = attn_pair(1, 2, fillers=[c11a, Q(1, 3), c11b,
                                                       K_(1, 3), b02(nc.sync),
                                                       OP(0, range(0, 4)),
                                                       c02a,
                                                       OP(0, range(4, 8)),
                                                       c02b],
                                        spread_ring=nc.sync)
            b03, c03a, c03b = attn_pair(0, 3, fillers=[b12(nc.sync),
                                                       V2(3, 0), c12a,
                                                       V2(3, 1), c12b,
                                                       OP(1, range(0, 4)),
                                                       OP(1, range(4, 8))],
                                        spread_ring=nc.sync)
            attn_pair(1, 3, fillers=[b03(nc.sync),
                                     OP(2, range(0, 3)),
                                     c03a,
                                     OP(2, range(3, 6)),
                                     c03b,
                                     OP(2, range(6, 8))],
                      tail=True)
            oproj_n(3, alternate=True, staged=False)

    nc.compile()
    return nc


def _get_program(causal: bool):
    if causal not in _programs:
        _programs[causal] = _build_program(causal)
    return _programs[causal]


def kernel(query, key, value, mask, Wq, Wk, Wv, Wo):
    global last_results
    from concourse.bass_utils import run_bass_kernel_spmd

    query = np.asarray(query, dtype=np.float32)
    key = np.asarray(key, dtype=np.float32)
    value = np.asarray(value, dtype=np.float32)
    Wq = np.asarray(Wq, dtype=np.float32)
    Wk = np.asarray(Wk, dtype=np.float32)
    Wv = np.asarray(Wv, dtype=np.float32)
    Wo = np.asarray(Wo, dtype=np.float32)
    m2d = np.asarray(mask).reshape(S, S).astype(bool)

    causal = bool(np.array_equal(m2d, np.tril(np.ones((S, S), dtype=bool))))
    nc = _get_program(causal)

    scale = 1.0 / math.sqrt(DH)
    WqT = np.ascontiguousarray((Wq * scale).T).astype(_BF16)
    WkT = np.ascontiguousarray(Wk.T).astype(_BF16)
    WvT = np.ascontiguousarray(Wv.T).astype(_BF16)
    WoT = np.ascontiguousarray(Wo.T).astype(_BF16)

    def tile_x(xTb):
        # [D, S] -> [QTILES, 128, KT*512]: group n holds token-columns
        # [512n, 512n+512) of all KT row-tiles, 8KB contiguous per partition
        return np.ascontiguousarray(
            xTb.reshape(KT, 128, QTILES, NQ).transpose(2, 1, 0, 3).reshape(
                QTILES, 128, KT * NQ
            )
        )

    def tile_w(wT):
        # [D, 256] -> [128, KT*256]
        return np.ascontiguousarray(
            wT.reshape(KT, 128, 256).transpose(1, 0, 2).reshape(128, KT * 256)
        )

    def tile_wo(woTs):
        # [256, D] -> [128, 2*D]: head h rows at 64*(h%2), chunk h//2
        o = np.zeros((128, 2, D), dtype=woTs.dtype)
        for h in range(GRP):
            base = 64 * (h % 2)
            o[base : base + 64, h // 2, :] = woTs[64 * h : 64 * h + 64, :]
        return np.ascontiguousarray(o.reshape(128, 2 * D))

    xT = {
        "qT": [tile_x(query[b].T.astype(_BF16)) for b in range(B)],
        "kT": [tile_x(key[b].T.astype(_BF16)) for b in range(B)],
        "vT": [tile_x(value[b].T.astype(_BF16)) for b in range(B)],
    }
    if not causal:
        maskT = np.ascontiguousarray(m2d.T).astype(_BF16)

    in_maps = []
    for c in range(NCORES):
        b, g = c // 4, c % 4
        sl = slice(256 * g, 256 * g + 256)
        im = {
            "qT": xT["qT"][b],
            "kT": xT["kT"][b],
            "vT": xT["vT"][b],
            "wqT": tile_w(WqT[:, sl]),
            "wkT": tile_w(WkT[:, sl]),
            "wvT": tile_w(WvT[:, sl]),
            "woT": tile_wo(WoT[sl, :]),
        }
        if not causal:
            im["maskT"] = maskT
        in_maps.append(im)

    trace = os.environ.get("KERNEL_PROFILE", "") == "1"
    res = run_bass_kernel_spmd(nc, in_maps, list(range(NCORES)), trace=trace)
    last_results = res

    outp = np.empty((B, S, D), dtype=np.float32)
    for b in range(B):
        acc = res.results[4 * b]["out"].astype(np.float32)
        for g in range(1, 4):
            acc = acc + res.results[4 * b + g]["out"].astype(np.float32)
        outp[b] = acc.T
    return outp


# revision 39
# speedup vs baseline: 1.1378x; 1.0131x over previous
"""Multi-head attention (B=2, S=2048, D=1024, H=16, causal) on 8 TRN2 NeuronCores.

Sharding: 8 cores = 2 batches x 4 head-groups (4 heads each).  Each core
computes the QKV projections for its head slice, causal attention for its 4
heads, and the partial output projection (input-dim slice of Wo).  The
all-reduce over head-groups happens at gather time on the host (sum of 4
partials per batch).  No device collectives; exec time is per-core.

Everything on device works in token-transposed layout ([feature, token]):
  scores^T[kv, q] = K_projT_tile^T @ Q_projT   (K = dh = 64)
  P = exp(scores^T)  (no max subtraction: scores ~ N(0,1), |s| < ~7)
  out^T = [ones|V|ones] band slices^T @ P      (ones half -> softmax denom,
                                                replicated across 64 rows by
                                                the PE at zero cycle cost)
  partial^T[dmodel, tok] = WoT_slice^T @ attn_out^T

v4 design notes (from perfetto/NTFF trace analysis; ~222us -> ~181us):
  - engine preamble is ~7us and aggregate DMA bandwidth ~260-280GB/s
    across all three rings (sync/scalar HWDGE + gpsimd SWDGE), so inputs
    are issued in strict global need-order with q0/k0 halved across two
    rings; PE warmup matmuls keep the clock ramp alive until wq+q0 land
    (~12us).  Mid-schedule is input-arrival-paced, not PE-paced.
  - q/k/v projection PSUM->SBUF copies run on the scalar engine (Copy is
    in every ACT table set; ACT is idle during the projection phase);
    oproj copies alternate DVE/ACT.  DVE keeps mask muls + normalize.
  - softmax normalize: the PE's ones-band replicates each head's
    denominator across a 64-row half.  Per pair: one evacuation copy
    releases the PSUM banks, the denominator rows spread over 128 lanes
    via two small DMA reshapes (reciprocal is ~6.4ns/elem/lane on one
    lane), then lane-parallel reciprocal, return DMAs, gpsimd
    partition_broadcasts, and two aligned DVE multiplies straight into
    attn2 (av rows coincide with attn2 rows by construction).  Stages are
    deferred 1-2 pairs (spread DMAs take 10-20us while bulk inputs
    stream) and placed in the mask-free first-4j-unit windows of their
    host pair so chain waits never head-of-line-block the DVE queue
    ahead of critical mask muls.
  - tail (last pair): no DMA round trip at all -- chunk-swapped copies
    put both replicated denominators in one [128, NQ] tile, ONE
    lane-parallel reciprocal covers both heads, partition_broadcast +
    stream_shuffle quadrant moves land the reciprocals on the av rows,
    and the multiplies read av straight from PSUM.
  - custom-DVE ops (reciprocal_approx_fast) and gpsimd elementwise are
    broken/slow on this toolchain (unit-tested); partition_broadcast only
    honors partition-0 input and base-0/full-128 output ranges.
"""

import math
import os

import numpy as np
import ml_dtypes

_BF16 = ml_dtypes.bfloat16

B, S, D = 2, 2048, 1024
H, DH = 16, 64
NCORES = 8
GRP = 4  # heads per core
KT = D // 128  # 8 k-tiles over d_model
NQ = 512  # q tile width
QTILES = S // NQ  # 4
KVTILES = S // 128  # 16

last_results = None

_programs = {}


def _build_program(causal: bool):
    import concourse.bass as bass
    import concourse.mybir as mybir
    import concourse.tile as tile
    from concourse import bacc

    f32 = mybir.dt.float32
    bf16 = mybir.dt.bfloat16
    Exp = mybir.ActivationFunctionType.Exp

    nc = bacc.Bacc(
        "TRN2",
        target_bir_lowering=False,
        debug=False,
        enable_asserts=False,
        num_devices=NCORES,
    )

    # all inputs host-pre-tiled so every DMA is one instruction with >=8KB
    # contiguous per-partition lines (1KB lines are packet-bound at ~97GB/s)
    qT = nc.dram_tensor("qT", [QTILES, 128, KT * NQ], bf16, kind="ExternalInput").ap()
    kT = nc.dram_tensor("kT", [QTILES, 128, KT * NQ], bf16, kind="ExternalInput").ap()
    vT = nc.dram_tensor("vT", [QTILES, 128, KT * NQ], bf16, kind="ExternalInput").ap()
    wqT = nc.dram_tensor("wqT", [128, KT * 256], bf16, kind="ExternalInput").ap()
    wkT = nc.dram_tensor("wkT", [128, KT * 256], bf16, kind="ExternalInput").ap()
    wvT = nc.dram_tensor("wvT", [128, KT * 256], bf16, kind="ExternalInput").ap()
    woT = nc.dram_tensor("woT", [128, 2 * D], bf16, kind="ExternalInput").ap()
    if not causal:
        maskT = nc.dram_tensor("maskT", [S, S], bf16, kind="ExternalInput").ap()
    out = nc.dram_tensor("out", [D, S], bf16, kind="ExternalOutput").ap()

    with tile.TileContext(nc) as tc:
        with (
            tc.tile_pool(name="persist", bufs=1) as sb,
            tc.tile_pool(name="stream", bufs=3) as stream,
            tc.tile_pool(name="psum", bufs=1, space="PSUM") as psum,
            tc.tile_pool(name="p_sb", bufs=6) as pbuf,
            tc.tile_pool(name="r_sb", bufs=2) as rpool,
            tc.tile_pool(name="m_sb", bufs=4) as mpool,
            tc.tile_pool(name="o_sb", bufs=4) as opool,
        ):
            # ---- persistent SBUF tensors ----
            wq_sb = sb.tile([128, KT, 256], bf16)
            wk_sb = sb.tile([128, KT, 256], bf16)
            wv_sb = sb.tile([128, KT, 256], bf16)
            wo2 = sb.tile([128, 2, D], bf16)  # head h at rows 64*(h%2), chunk h//2
            qproj = sb.tile([128, 2, S], bf16)
            kproj = sb.tile([128, 2, S], bf16)
            attn2 = sb.tile([128, 2, S], bf16)  # head h at rows 64*(h%2), chunk h//2

            # V with 64-wide ones flanks: per (kv-tile, head g): cols 0..63 =
            # ones, 64..127 = V, 128..191 = ones.  Stationary [64:192] ->
            # av rows 0..63 + denom replicated on rows 64..127 (even heads);
            # [0:128] -> denom replicated rows 0..63 + av rows 64..127 (odd
            # heads).  Replication is free on the PE (M=128 vs 65 costs no
            # cycles) and lets the tail chain run a direct lane-parallel
            # reciprocal with no DMA round trip.
            vproj = sb.tile([128, KVTILES, GRP, 192], bf16)

            # ---- gpsimd queue: tiny warm memset FIRST so warmup matmuls
            # are unblocked right after the engine preamble (~5us).
            warm = sb.tile([128, 256], bf16)
            nc.gpsimd.memset(warm[:], 0.0)

            # PE warmup: dummy matmuls fill the DMA-paced startup window and
            # keep the clock ramp going until real matmuls start.
            _warm_ctr = [0]

            def warmup(k):
                for _ in range(k):
                    w = _warm_ctr[0]
                    _warm_ctr[0] += 1
                    wp = psum.tile(
                        [128, 256], f32, tag="mm", bufs=2, name=f"warm{w}"
                    )
                    nc.tensor.matmul(
                        wp[:], warm[:, 0:128], warm[:], start=True, stop=True
                    )

            # ---- input DMA prologue.  Aggregate DMA bandwidth is ~280GB/s
            # shared across the three rings (sync HWDGE, scalar HWDGE,
            # gpsimd SWDGE), so what matters is global need-order: wq+q0
            # first (first projection at ~12us), then k0+wk, wv+v0, then the
            # later groups.  Each 1MB token group is split into two 0.5MB
            # halves on different rings to keep arrival granularity fine.
            qg, kg, vg = [], [], []

            def _mktile(dst_list, tag):
                n = len(dst_list)
                t = stream.tile(
                    [128, KT, NQ], bf16, tag=tag, bufs=3, name=f"{tag}{n}"
                )
                dst_list.append(t)
                return t

            def _half(t, src, n, half, eng):
                h = KT // 2
                sl = slice(half * h, (half + 1) * h)
                eng.dma_start(t[:, sl, :], src[n][:, half * h * NQ : (half + 1) * h * NQ])

            # global need order: wq, q0 | wk, k0 | v0, q1 | k1, v1 | q2, k2 |
            # q3, k3 | v2, v3 (wo anywhere past ~30us).  q0/k0 are halved
            # across sync+gpsimd so the first projections start ~12us; the
            # rest are full 1MB transfers (fewer completion bubbles on the 8
            # DMA semaphore lanes).  Ring FIFO order matches need order.
            nc.scalar.dma_start(wq_sb[:], wqT[:])
            t = _mktile(qg, "qg")
            _half(t, qT, 0, 0, nc.sync)
            _half(t, qT, 0, 1, nc.gpsimd)
            t = _mktile(kg, "kg")
            _half(t, kT, 0, 0, nc.sync)
            _half(t, kT, 0, 1, nc.gpsimd)
            nc.scalar.dma_start(wk_sb[:], wkT[:])
            nc.scalar.dma_start(wv_sb[:], wvT[:])
            t = _mktile(vg, "vg")
            nc.sync.dma_start(t[:], vT[0])
            t = _mktile(qg, "qg")
            nc.gpsimd.dma_start(t[:], qT[1])
            t = _mktile(kg, "kg")
            nc.sync.dma_start(t[:], kT[1])
            t = _mktile(vg, "vg")
            nc.sync.dma_start(t[:], vT[1])
            t = _mktile(qg, "qg")
            nc.gpsimd.dma_start(t[:], qT[2])
            t = _mktile(kg, "kg")
            nc.gpsimd.dma_start(t[:], kT[2])
            nc.scalar.dma_start(wo2[:], woT[:])
            t = _mktile(qg, "qg")
            nc.sync.dma_start(t[:], qT[3])
            t = _mktile(kg, "kg")
            nc.gpsimd.dma_start(t[:], kT[3])
            t = _mktile(vg, "vg")
            nc.gpsimd.dma_start(t[:], vT[2])
            t = _mktile(vg, "vg")
            nc.sync.dma_start(t[:], vT[3])

            # gpsimd setup after its DMA issues: causal mask (needed ~28us)
            if causal:
                # single 128x128 causal block: keep where q_local >= kv_local
                mask128 = sb.tile([128, 128], bf16)
                nc.gpsimd.memset(mask128[:], 1.0)
                nc.gpsimd.affine_select(
                    out=mask128[:],
                    in_=mask128[:],
                    compare_op=mybir.AluOpType.is_ge,
                    fill=0.0,
                    base=0,
                    pattern=[[1, 128]],
                    channel_multiplier=-1,
                )
            # ones flanks on gpsimd (its queue is idle once the DMA
            # issues are out; done by ~20us, first AV needs them ~28us)
            nc.gpsimd.memset(vproj[:, :, :, 0:64], 1.0)
            nc.gpsimd.memset(vproj[:, :, :, 128:192], 1.0)

            warmup(24)

            def qkproj(which, m2, n):
                w_sb = wq_sb if which == "q" else wk_sb
                xt = (qg if which == "q" else kg)[n]
                proj = qproj if which == "q" else kproj
                ps = psum.tile([128, NQ], f32, tag="mm", bufs=2)
                for kt in range(KT):
                    nc.tensor.matmul(
                        ps[:],
                        w_sb[:, kt, 128 * m2 : 128 * m2 + 128],
                        xt[:, kt, :],
                        start=(kt == 0),
                        stop=(kt == KT - 1),
                    )
                nc.scalar.copy(proj[:, m2, NQ * n : NQ * n + NQ], ps[:])

            def vproj_tile(mt):
                vt = vg[mt // 4]
                col = 128 * (mt % 4)
                ps = psum.tile([128, 256], f32, tag="mm", bufs=2)
                for kt in range(KT):
                    nc.tensor.matmul(
                        ps[:],
                        vt[:, kt, col : col + 128],
                        wv_sb[:, kt, :],
                        start=(kt == 0),
                        stop=(kt == KT - 1),
                    )
                nc.scalar.copy(
                    vproj[:, mt, :, 64:128],
                    ps[:].rearrange("p (h d) -> p h d", h=GRP),
                )

            # attention state per (c2): av psum tile + per-tile units
            def attn_units(c2, j, avt):
                """Yield per-kv-tile unit emitters for head pair c2, q-tile j."""
                ktiles = 4 * j + 4 if causal else KVTILES
                p2s = {}

                def sc_exp(t):
                    d = t - 4 * j
                    off = 128 * d if (causal and d >= 0) else 0
                    sc = psum.tile([128, 2, NQ], f32, tag="sc", bufs=2)
                    for i in range(2):
                        base = 64 * i
                        nc.tensor.matmul(
                            sc[:, i, off:NQ],
                            kproj[base : base + 64, c2, 128 * t : 128 * t + 128],
                            qproj[base : base + 64, c2, NQ * j + off : NQ * j + NQ],
                            start=True,
                            stop=True,
                        )
                    p2 = pbuf.tile([128, 2, NQ], bf16, tag="p")
                    p2s[t] = p2
                    nc.scalar.activation(p2[:, :, off:NQ], sc[:, :, off:NQ], Exp)

                def mask_av(t):
                    d = t - 4 * j
                    off = 128 * d if (causal and d >= 0) else 0
                    p2 = p2s[t]
                    if causal:
                        if d >= 0:
                            for i in range(2):
                                nc.vector.tensor_mul(
                                    p2[:, i, off : off + 128],
                                    p2[:, i, off : off + 128],
                                    mask128[:],
                                )
                    else:
                        mt_t = mpool.tile([128, NQ], bf16, tag="mt")
                        nc.sync.dma_start(
                            mt_t[:],
                            maskT[128 * t : 128 * t + 128, NQ * j : NQ * j + NQ],
                        )
                        for i in range(2):
                            nc.vector.tensor_mul(p2[:, i, :], p2[:, i, :], mt_t[:])
                    for i in range(2):
                        g = 2 * c2 + i
                        lo = 64 - 64 * i  # i=0 -> [64:192], i=1 -> [0:128]
                        nc.tensor.matmul(
                            avt[:, i, off:NQ],
                            vproj[:, t, g, lo : lo + 128],
                            p2[:, i, off:NQ],
                            start=(t == 0),
                            stop=(t == ktiles - 1),
                        )

                def unit(t):
                    sc_exp(t)
                    mask_av(t)

                return ktiles, sc_exp, mask_av, unit

            def normalize_evac(avt, ring):
                # One copy releases the av PSUM banks; then the denominator
                # rows (row 64 chunk0 / row 0 chunk1) are spread over all 128
                # lanes via DMA reshapes (reciprocal is ~6.4ns/elem/lane).
                # The remaining stages run as fillers of LATER pairs, each
                # emitted only once its dependency has had time to complete
                # -- chain ops must never head-of-line-block an engine queue
                # that critical ops (mask muls) share.
                src = rpool.tile([128, 2, NQ], f32, tag="avs", bufs=3)
                nc.vector.tensor_copy(src[:], avt[:])
                rq = rpool.tile([128, 8], f32, tag="rq", bufs=3)
                ring.dma_start(rq[:, 0:4], src[64:65, 0, :])
                ring.dma_start(rq[:, 4:8], src[0:1, 1, :])
                return src, rq

            def chain_b(src_rq, ring):
                # stage B: lane-parallel reciprocal + return DMAs
                src, rq = src_rq
                rqr = rpool.tile([128, 8], f32, tag="rqr")
                nc.vector.reciprocal(rqr[:], rq[:])
                rz = rpool.tile([1, 2, NQ], f32, tag="rz")
                ring.dma_start(rz[0:1, 0, :], rqr[:, 0:4])
                ring.dma_start(rz[0:1, 1, :], rqr[:, 4:8])
                return rz

            def chain_c1(rz):
                # stage C1: broadcast across partitions on the idle gpsimd
                rb = rpool.tile([128, 2, NQ], f32, tag="rb")
                nc.gpsimd.partition_broadcast(rb[:, 1, :], rz[0:1, 1, :],
                                              channels=128)
                nc.gpsimd.partition_broadcast(rb[0:64, 0, :], rz[0:1, 0, :],
                                              channels=64)
                return rb

            def chain_c2(c2, j, src_rq, rb):
                # stage C2: final multiplies on DVE, emitted a few filler
                # slots after C1 so the pbcasts are done by then
                src, _ = src_rq
                nc.vector.tensor_mul(
                    attn2[64:128, c2, NQ * j : NQ * j + NQ],
                    src[64:128, 1, :],
                    rb[64:128, 1, :],
                )
                nc.vector.tensor_mul(
                    attn2[0:64, c2, NQ * j : NQ * j + NQ],
                    src[0:64, 0, :],
                    rb[0:64, 0, :],
                )

            def normalize_tail(c2, j, avt):
                # Tail variant, no DMA and no full evacuation: copy just the
                # two PE-replicated denominator halves (chunk-swapped) into
                # one [128, NQ] tile, ONE lane-parallel reciprocal covers
                # both heads, then pbcast (1/den1) + stream_shuffle quadrant
                # moves (1/den0) land them on the av rows; the multiplies
                # read av straight out of PSUM (nobody needs the banks).
                dcomb = rpool.tile([128, NQ], f32, tag="dcomb", bufs=1)
                nc.vector.tensor_copy(dcomb[0:64, :], avt[0:64, 1, :])
                nc.vector.tensor_copy(dcomb[64:128, :], avt[64:128, 0, :])
                rr = rpool.tile([128, NQ], f32, tag="rr", bufs=1)
                nc.vector.reciprocal(rr[:], dcomb[:])
                rbt = rpool.tile([128, NQ], f32, tag="trb", bufs=1)
                nc.gpsimd.partition_broadcast(rbt[:, :], rr[0:1, :],
                                              channels=128)
                tsh = rpool.tile([64, NQ], f32, tag="tsh", bufs=1)
                ident = list(range(32))
                nc.vector.stream_shuffle(tsh[0:32, :], rr[64:96, :], ident)
                nc.vector.stream_shuffle(tsh[32:64, :], rr[96:128, :], ident)
                nc.vector.tensor_mul(
                    attn2[0:64, c2, NQ * j : NQ * j + NQ],
                    avt[0:64, 0, :],
                    tsh[0:64, :],
                )
                nc.vector.tensor_mul(
                    attn2[64:128, c2, NQ * j : NQ * j + NQ],
                    avt[64:128, 1, :],
                    rbt[64:128, :],
                )

            def attn_pair(c2, j, fillers=(), spread_ring=None, tail=False):
                """Emit one head-pair x q-tile attention, interleaving filler
                emitters (projection groups, oproj chunks, deferred normalize
                chain stages) between kv-tile units.  Returns (b, c1, c2)
                stage emitters for the deferred normalize; the evacuation +
                denominator-spread DMAs are emitted immediately so the PSUM
                banks recycle and the transfers complete during later
                pairs."""
                avt = psum.tile([128, 2, NQ], f32, tag="av", bufs=1,
                                name=f"av{c2}{j}")
                ktiles, sc_exp, mask_av, unit = attn_units(c2, j, avt)

                def spread(emitters, fill):
                    nf, nu = len(fill), len(emitters)
                    fi = 0
                    for ui, u in enumerate(emitters):
                        u()
                        while fi < nf and fi * nu <= (ui + 1) * nf - 1:
                            fill[fi]()
                            fi += 1
                    while fi < nf:
                        fill[fi]()
                        fi += 1

                spread([lambda t=t: unit(t) for t in range(ktiles)],
                       list(fillers))
                if tail:
                    normalize_tail(c2, j, avt)
                    return None, None, None
                src_rq = normalize_evac(avt, spread_ring)

                state = {}

                def b(ring):
                    def f():
                        state["rz"] = chain_b(src_rq, ring)
                    return f

                def stage_c1():
                    state["rb"] = chain_c1(state["rz"])

                def stage_c2():
                    chain_c2(c2, j, src_rq, state["rb"])

                return b, stage_c1, stage_c2

            def oproj_m(n, m, act_copy=False):
                ps = psum.tile([128, NQ], f32, tag="mm", bufs=2)
                for c2 in range(2):
                    nc.tensor.matmul(
                        ps[:],
                        wo2[:, c2, 128 * m : 128 * m + 128],
                        attn2[:, c2, NQ * n : NQ * n + NQ],
                        start=(c2 == 0),
                        stop=(c2 == 1),
                    )
                # per-m out DMAs: a staged 1MB [p m q] DMA has 1KB dram lines
                # (packet-bound ~97GB/s => ~11us) and ring-order-blocks later
                # sync DMAs (tail chain spreads, final outs) -- keep 128KB
                # transfers that pipeline behind each copy instead
                ot = opool.tile([128, NQ], bf16, tag="otm", bufs=4)
                if act_copy:
                    nc.scalar.copy(ot[:], ps[:])
                else:
                    nc.vector.tensor_copy(ot[:], ps[:])
                nc.sync.dma_start(
                    out[128 * m : 128 * m + 128, NQ * n : NQ * n + NQ],
                    ot[:],
                )

            def oproj_n(n, alternate=False):
                # alternate=True splits the PSUM->SBUF copies between DVE and
                # ACT -- used for the tail block where ACT has no exp left.
                for m in range(D // 128):
                    oproj_m(n, m, act_copy=alternate and (m % 2 == 1))

            def Q(m2, n):
                return lambda: qkproj("q", m2, n)

            def K_(m2, n):
                return lambda: qkproj("k", m2, n)

            def V2(n, half):
                mts = range(4 * n + 2 * half, 4 * n + 2 * half + 2)
                def f():
                    for mt in mts:
                        vproj_tile(mt)
                return f

            def OP(n, ms):
                def f():
                    for m in ms:
                        # alternate PSUM->SBUF copies between DVE and ACT
                        oproj_m(n, m, act_copy=(m % 2 == 1))
                return f

            # ---- emission schedule ----
            # PE is the binding engine: keep it dense.  Fillers are spread
            # between attention units; a pair's fillers must not be among its
            # own dependencies EXCEPT vproj fillers, which land earlier in the
            # unit loop than the first unit that reads them (verified against
            # the spread formula).  Deferred normalize chains run as fillers
            # of the following pair, on the scalar ring while the sync ring
            # still streams inputs (~first 55us), sync after.  Extra warmup
            # blocks cover the k0/v0 DMA arrival gaps.  The small (1,0) pair
            # runs last to keep the post-exp tail short.
            # Early chains (pairs ending before the ~60us bulk-input drain)
            # have their B stage deferred TWO pairs so the spread DMAs never
            # head-of-line-block the DVE queue; later chains use one-pair
            # staging (DMA latency drops once the bulk transfers finish).
            Q(0, 0)()
            Q(1, 0)()
            warmup(16)
            K_(0, 0)()
            K_(1, 0)()
            warmup(12)
            V2(0, 0)()
            V2(0, 1)()
            # Chain DMA latency is 10-20us while bulk inputs stream
            # (<60us), so the first two chains defer their B stage two pairs;
            # later chains use one-pair staging.  Stages are placed in the
            # mask-free first-4j-unit windows of their host pair so they
            # never head-of-line-block the DVE queue ahead of mask muls.
            b00, c00a, c00b = attn_pair(0, 0, fillers=[Q(0, 1), Q(1, 1),
                                                       K_(0, 1), K_(1, 1)],
                                        spread_ring=nc.scalar)
            b10, c10a, c10b = attn_pair(1, 0, fillers=[Q(1, 2)],
                                        spread_ring=nc.scalar)
            b01, c01a, c01b = attn_pair(0, 1, fillers=[V2(1, 0), V2(1, 1),
                                                       Q(0, 2), K_(0, 2)],
                                        spread_ring=nc.scalar)
            b11, c11a, c11b = attn_pair(1, 1, fillers=[b00(nc.scalar),
                                                       Q(0, 3), c00a,
                                                       K_(1, 2), c00b,
                                                       b10(nc.scalar)],
                                        spread_ring=nc.scalar)
            b02, c02a, c02b = attn_pair(0, 2, fillers=[c10a, V2(2, 0), c10b,
                                                       b01(nc.scalar),
                                                       V2(2, 1), c01a,
                                                       K_(0, 3), c01b,
                                                       b11(nc.scalar)],
                                        spread_ring=nc.sync)
            b12, c12a, c12b = attn_pair(1, 2, fillers=[c11a, Q(1, 3), c11b,
                                                       K_(1, 3), b02(nc.sync),
                                                       OP(0, range(0, 4)),
                                                       c02a,
                                                       OP(0, range(4, 8)),
                                                       c02b],
                                        spread_ring=nc.sync)
            b03, c03a, c03b = attn_pair(0, 3, fillers=[b12(nc.sync),
                                                       V2(3, 0), c12a,
                                                       V2(3, 1), c12b,
                                                       OP(1, range(0, 4)),
                                                       OP(1, range(4, 8))],
                                        spread_ring=nc.sync)
            attn_pair(1, 3, fillers=[b03(nc.sync),
                                     OP(2, range(0, 3)),
                                     c03a,
                                     OP(2, range(3, 6)),
                                     c03b,
                                     OP(2, range(6, 8))],
                      tail=True)
            oproj_n(3, alternate=True)

    nc.compile()
    return nc


def _get_program(causal: bool):
    if causal not in _programs:
        _programs[causal] = _build_program(causal)
    return _programs[causal]


def kernel(query, key, value, mask, Wq, Wk, Wv, Wo):
    global last_results
    from concourse.bass_utils import run_bass_kernel_spmd

    query = np.asarray(query, dtype=np.float32)
    key = np.asarray(key, dtype=np.float32)
    value = np.asarray(value, dtype=np.float32)
    Wq = np.asarray(Wq, dtype=np.float32)
    Wk = np.asarray(Wk, dtype=np.float32)
    Wv = np.asarray(Wv, dtype=np.float32)
    Wo = np.asarray(Wo, dtype=np.float32)
    m2d = np.asarray(mask).reshape(S, S).astype(bool)

    causal = bool(np.array_equal(m2d, np.tril(np.ones((S, S), dtype=bool))))
    nc = _get_program(causal)

    scale = 1.0 / math.sqrt(DH)
    WqT = np.ascontiguousarray((Wq * scale).T).astype(_BF16)
    WkT = np.ascontiguousarray(Wk.T).astype(_BF16)
    WvT = np.ascontiguousarray(Wv.T).astype(_BF16)
    WoT = np.ascontiguousarray(Wo.T).astype(_BF16)

    def tile_x(xTb):
        # [D, S] -> [QTILES, 128, KT*512]: group n holds token-columns
        # [512n, 512n+512) of all KT row-tiles, 8KB contiguous per partition
        return np.ascontiguousarray(
            xTb.reshape(KT, 128, QTILES, NQ).transpose(2, 1, 0, 3).reshape(
                QTILES, 128, KT * NQ
            )
        )

    def tile_w(wT):
        # [D, 256] -> [128, KT*256]
        return np.ascontiguousarray(
            wT.reshape(KT, 128, 256).transpose(1, 0, 2).reshape(128, KT * 256)
        )

    def tile_wo(woTs):
        # [256, D] -> [128, 2*D]: head h rows at 64*(h%2), chunk h//2
        o = np.zeros((128, 2, D), dtype=woTs.dtype)
        for h in range(GRP):
            base = 64 * (h % 2)
            o[base : base + 64, h // 2, :] = woTs[64 * h : 64 * h + 64, :]
        return np.ascontiguousarray(o.reshape(128, 2 * D))

    xT = {
        "qT": [tile_x(query[b].T.astype(_BF16)) for b in range(B)],
        "kT": [tile_x(key[b].T.astype(_BF16)) for b in range(B)],
        "vT": [tile_x(value[b].T.astype(_BF16)) for b in range(B)],
    }
    if not causal:
        maskT = np.ascontiguousarray(m2d.T).astype(_BF16)

    in_maps = []
    for c in range(NCORES):
        b, g = c // 4, c % 4
        sl = slice(256 * g, 256 * g + 256)
        im = {
            "qT": xT["qT"][b],
            "kT": xT["kT"][b],
            "vT": xT["vT"][b],
            "wqT": tile_w(WqT[:, sl]),
            "wkT": tile_w(WkT[:, sl]),
            "wvT": tile_w(WvT[:, sl]),
            "woT": tile_wo(WoT[sl, :]),
        }
        if not causal:
            im["maskT"] = maskT
        in_maps.append(im)

    trace = os.environ.get("KERNEL_PROFILE", "") == "1"
    res = run_bass_kernel_spmd(nc, in_maps, list(range(NCORES)), trace=trace)
    last_results = res

    outp = np.empty((B, S, D), dtype=np.float32)
    for b in range(B):
        acc = res.results[4 * b]["out"].astype(np.float32)
        for g in range(1, 4):
            acc = acc + res.results[4 * b + g]["out"].astype(np.float32)
        outp[b] = acc.T
    return outp
